# revision 1
# baseline (speedup 1.0000x reference)
"""Trainium2 Bass kernel for nn_DKNN (sparse attention with per-row top-k mask).

Computation (see reference.py):
    ae_q  = MLP(feat_q)   ae_kv = MLP(feat_kv)        (3-layer, PReLU(0.25))
    q_in  = 0.5*ae_q + 0.5*pe_q ; k_in = 0.5*ae_kv + 0.5*pe_kv
    query = q_in @ Wq + q_in ;    key  = k_in @ Wk + k_in
    att   = (query @ key.T) / 16                       [8192, 4096]
    S     = (pe_q @ pe_kv.T) / 16
    thresh= 64th largest of S per row
    out   = where(S < thresh, 0, att)

Sharding: 8 cores, each handles 1024 query rows; kv + weights replicated.

Per-core kernel (all in "transposed" space, d_model on partitions):
  - PE transposes inputs; MLP/projections run as f32r matmuls (TF32-grade,
    only affects att values ~1e-4 rel); pe_sims runs in exact fp32.
  - residuals folded into host-precomputed Wq+I / Wk+I; 1/16 folded into
    q-side scales.
  - top-64/row: 64x max8 over 64-wide chunks -> 512 candidates; then
    8 rounds max8+match_replace -> exact 64th-largest of candidates.
  - mask applied by DVE multiply during att PSUM drain.
"""

import numpy as np

import concourse.bass as bass
import concourse.mybir as mybir
import concourse.tile as tile
from concourse import bacc, masks
from concourse.bass_utils import run_bass_kernel_spmd

F32 = mybir.dt.float32
F32R = mybir.dt.float32r

N_CORES = 8
BQ = 8192
NK = 4096
D_IN = 128
D_MODEL = 256
TOP_K = 64
QR = BQ // N_CORES          # query rows per core = 1024
N_TILES = QR // 128         # 8 q-tiles of 128 rows per core
CHUNK = 64                  # candidate chunk width for topk phase 1
NCH = NK // CHUNK           # 64 chunks
NEG = -1e30

_CACHE = {}


def _build(alpha: float, b3_zero: bool, main_reps: int = 1):
    nc = bacc.Bacc("TRN2", target_bir_lowering=False, debug=False)

    feat_q = nc.dram_tensor("feat_q", [QR, D_IN], F32, kind="ExternalInput")
    pe_q = nc.dram_tensor("pe_q", [QR, D_MODEL], F32, kind="ExternalInput")
    feat_kv = nc.dram_tensor("feat_kv", [NK, D_IN], F32, kind="ExternalInput")
    pe_kv = nc.dram_tensor("pe_kv", [NK, D_MODEL], F32, kind="ExternalInput")
    W1 = nc.dram_tensor("W1", [D_IN, D_MODEL], F32, kind="ExternalInput")
    W2 = nc.dram_tensor("W2", [D_MODEL, D_MODEL], F32, kind="ExternalInput")
    W3kv = nc.dram_tensor("W3kv", [D_MODEL, D_MODEL], F32, kind="ExternalInput")  # 0.5*W3
    W3q = nc.dram_tensor("W3q", [D_MODEL, D_MODEL], F32, kind="ExternalInput")    # W3/32
    Wqp = nc.dram_tensor("Wqp", [D_MODEL, D_MODEL], F32, kind="ExternalInput")    # Wq+I
    Wkp = nc.dram_tensor("Wkp", [D_MODEL, D_MODEL], F32, kind="ExternalInput")    # Wk+I
    b1 = nc.dram_tensor("b1", [128, 2], F32, kind="ExternalInput")
    b2 = nc.dram_tensor("b2", [128, 2], F32, kind="ExternalInput")
    b3kv = nc.dram_tensor("b3kv", [128, 2], F32, kind="ExternalInput")  # 0.5*b3
    b3q = nc.dram_tensor("b3q", [128, 2], F32, kind="ExternalInput")    # b3/32
    out = nc.dram_tensor("out", [QR, NK], F32, kind="ExternalOutput")

    AF = mybir.ActivationFunctionType

    with tile.TileContext(nc) as tc:
        with tc.tile_pool(name="wgt", bufs=1) as wgt, \
             tc.tile_pool(name="persist", bufs=1) as persist:

            # ---------------- weights / biases / identity ----------------
            ident = wgt.tile([128, 128], F32, tag="ident")
            masks.make_identity(nc, ident[:])

            with tc.tile_pool(name="wraw", bufs=2) as wraw:
                def load_w_f32r(dram, kchunks, tag):
                    tiles = []
                    for k in range(kchunks):
                        t32 = wraw.tile([128, D_MODEL], F32, tag="wstage",
                                        name=f"{tag}{k}_raw")
                        nc.sync.dma_start(t32[:], dram.ap()[k * 128:(k + 1) * 128, :])
                        tr = wgt.tile([128, D_MODEL], F32R, tag=f"{tag}{k}",
                                      name=f"{tag}{k}")
                        nc.vector.tensor_copy(tr[:], t32[:])
                        tiles.append(tr)
                    return tiles

                w1 = load_w_f32r(W1, 1, "w1")
                w2 = load_w_f32r(W2, 2, "w2")
                w3kv = load_w_f32r(W3kv, 2, "w3kv")
                w3q = load_w_f32r(W3q, 2, "w3q")
                wqp = load_w_f32r(Wqp, 2, "wqp")
                wkp = load_w_f32r(Wkp, 2, "wkp")

            def load_bias(dram, tag):
                t = wgt.tile([128, 2], F32, tag=tag, name=tag)
                nc.sync.dma_start(t[:], dram.ap())
                return t

            b1t = load_bias(b1, "b1t")
            b2t = load_bias(b2, "b2t")
            b3kvt = load_bias(b3kv, "b3kvt")
            b3qt = load_bias(b3q, "b3qt")

            # persistent per-core tensors
            pekv_h = [persist.tile([128, NK], F32, tag=f"pekvh{k}", name=f"pekvh{k}")
                      for k in range(2)]
            keyT = [persist.tile([128, NK], F32R, tag=f"keyT{k}", name=f"keyT{k}")
                    for k in range(2)]
            pq8 = [persist.tile([128, QR], F32, tag=f"pq8_{k}", name=f"pq8_{k}")
                   for k in range(2)]
            qT = [persist.tile([128, QR], F32R, tag=f"qT{k}", name=f"qT{k}")
                  for k in range(2)]

            # ---------------- transpose helper ----------------
            # Groups of 4 row-blocks -> one [128,512] PSUM bank -> 1 ACT drain
            # per (k-chunk, target).
            def transpose_in(dram, rows, cols, drains, ldpool, tpool):
                """drains: {k: [(dst_tile, scale, bias_ap_or_None)]};
                dst gets [128(dmodel chunk k), rows]."""
                ngrp = rows // 512
                for g in range(ngrp):
                    st = ldpool.tile([128, 4, cols], F32, tag="tstage", name="tstage")
                    src = dram.ap()[g * 512:(g + 1) * 512, :]
                    nc.sync.dma_start(
                        st[:], src.rearrange("(j p) c -> p j c", p=128))
                    for k in range(cols // 128):
                        ps = tpool.tile([128, 512], F32, tag="tpsum", name="tpsum")
                        for j in range(4):
                            nc.tensor.transpose(
                                ps[:, j * 128:(j + 1) * 128],
                                st[:, j, k * 128:(k + 1) * 128], ident[:])
                        for (dst, scale, bias_ap) in drains[k]:
                            csl = slice(g * 512, (g + 1) * 512)
                            if bias_ap is None:
                                nc.scalar.activation(dst[:, csl], ps[:],
                                                     AF.Copy, bias=0.0, scale=scale)
                            else:
                                nc.scalar.activation(dst[:, csl], ps[:],
                                                     AF.Identity, bias=bias_ap,
                                                     scale=scale)

            # ---------------- kv side: transposes + MLP -> keyT ----------------
            NB = NK // 512
            with tc.tile_pool(name="kvsb", bufs=2) as kvsb, \
                 tc.tile_pool(name="kvps", bufs=4, space="PSUM") as kvps, \
                 tc.tile_pool(name="kvtld", bufs=3) as kvtld, \
                 tc.tile_pool(name="kvtps", bufs=4, space="PSUM") as kvtps:

                transpose_in(pe_kv, NK, D_MODEL,
                             {0: [(pekv_h[0], 0.5, None)],
                              1: [(pekv_h[1], 0.5, None)]},
                             kvtld, kvtps)
                fkT = kvsb.tile([128, NK], F32R, tag="fkT", bufs=1)
                transpose_in(feat_kv, NK, D_IN, {0: [(fkT, 1.0, None)]},
                             kvtld, kvtps)

                # layer 1
                h1 = [kvsb.tile([128, NK], F32R, tag="h1kin", name=f"h1_{m}")
                      for m in range(2)]
                for m in range(2):
                    for n in range(NB):
                        ps = kvps.tile([128, 512], F32, tag="mlp", name="mlp_ps")
                        nc.tensor.matmul(ps[:], w1[0][:, m * 128:(m + 1) * 128],
                                         fkT[:, n * 512:(n + 1) * 512],
                                         start=True, stop=True)
                        nc.scalar.activation(h1[m][:, n * 512:(n + 1) * 512], ps[:],
                                             AF.Prelu, bias=b1t[:, m:m + 1],
                                             scale=1.0, alpha=alpha)
                # layer 2
                h2 = [kvsb.tile([128, NK], F32R, tag="h2", name=f"h2_{m}")
                      for m in range(2)]
                for m in range(2):
                    for n in range(NB):
                        ps = kvps.tile([128, 512], F32, tag="mlp", name="mlp_ps")
                        for k in range(2):
                            nc.tensor.matmul(ps[:], w2[k][:, m * 128:(m + 1) * 128],
                                             h1[k][:, n * 512:(n + 1) * 512],
                                             start=(k == 0), stop=(k == 1))
                        nc.scalar.activation(h2[m][:, n * 512:(n + 1) * 512], ps[:],
                                             AF.Prelu, bias=b2t[:, m:m + 1],
                                             scale=1.0, alpha=alpha)
                # layer 3 + k_in ; kin reuses h1's slots (tag h1kin)
                kin = [kvsb.tile([128, NK], F32R, tag="h1kin", name=f"kin{m}")
                       for m in range(2)]
                for m in range(2):
                    for n in range(NB):
                        ps = kvps.tile([128, 512], F32, tag="mlp", name="mlp_ps")
                        for k in range(2):
                            nc.tensor.matmul(ps[:], w3kv[k][:, m * 128:(m + 1) * 128],
                                             h2[k][:, n * 512:(n + 1) * 512],
                                             start=(k == 0), stop=(k == 1))
                        sl = slice(n * 512, (n + 1) * 512)
                        if b3_zero:
                            nc.vector.tensor_add(kin[m][:, sl], ps[:], pekv_h[m][:, sl])
                        else:
                            tmp = kvsb.tile([128, 512], F32, tag="aetmp",
                                            name="aetmp", bufs=3)
                            nc.scalar.activation(tmp[:], ps[:], AF.Identity,
                                                 bias=b3kvt[:, m:m + 1], scale=1.0)
                            nc.vector.tensor_add(kin[m][:, sl], tmp[:], pekv_h[m][:, sl])
                # key projection
                for m in range(2):
                    for n in range(NB):
                        ps = kvps.tile([128, 512], F32, tag="mlp", name="mlp_ps")
                        for k in range(2):
                            nc.tensor.matmul(ps[:], wkp[k][:, m * 128:(m + 1) * 128],
                                             kin[k][:, n * 512:(n + 1) * 512],
                                             start=(k == 0), stop=(k == 1))
                        nc.scalar.activation(keyT[m][:, n * 512:(n + 1) * 512], ps[:],
                                             AF.Copy, bias=0.0, scale=1.0)

            # ---------------- q side: transposes + MLP -> qT ----------------
            QB = QR // 512
            with tc.tile_pool(name="qsb", bufs=2) as qsb, \
                 tc.tile_pool(name="qps", bufs=4, space="PSUM") as qps, \
                 tc.tile_pool(name="qtld", bufs=3) as qtld, \
                 tc.tile_pool(name="qtps", bufs=4, space="PSUM") as qtps:

                pq32 = [qsb.tile([128, QR], F32, tag=f"pq32_{k}", name=f"pq32_{k}",
                                 bufs=1) for k in range(2)]
                transpose_in(pe_q, QR, D_MODEL,
                             {0: [(pq8[0], 1.0 / 8, None), (pq32[0], 1.0 / 32, None)],
                              1: [(pq8[1], 1.0 / 8, None), (pq32[1], 1.0 / 32, None)]},
                             qtld, qtps)
                fqT = qsb.tile([128, QR], F32R, tag="fqT", bufs=1)
                transpose_in(feat_q, QR, D_IN, {0: [(fqT, 1.0, None)]}, qtld, qtps)

                h1q = [qsb.tile([128, QR], F32R, tag="h1qin", name=f"h1q{m}")
                       for m in range(2)]
                for m in range(2):
                    for n in range(QB):
                        ps = qps.tile([128, 512], F32, tag="qmlp", name="qmlp_ps")
                        nc.tensor.matmul(ps[:], w1[0][:, m * 128:(m + 1) * 128],
                                         fqT[:, n * 512:(n + 1) * 512],
                                         start=True, stop=True)
                        nc.scalar.activation(h1q[m][:, n * 512:(n + 1) * 512], ps[:],
                                             AF.Prelu, bias=b1t[:, m:m + 1],
                                             scale=1.0, alpha=alpha)
                h2q = [qsb.tile([128, QR], F32R, tag="h2q", name=f"h2q{m}")
                       for m in range(2)]
                for m in range(2):
                    for n in range(QB):
                        ps = qps.tile([128, 512], F32, tag="qmlp", name="qmlp_ps")
                        for k in range(2):
                            nc.tensor.matmul(ps[:], w2[k][:, m * 128:(m + 1) * 128],
                                             h1q[k][:, n * 512:(n + 1) * 512],
                                             start=(k == 0), stop=(k == 1))
                        nc.scalar.activation(h2q[m][:, n * 512:(n + 1) * 512], ps[:],
                                             AF.Prelu, bias=b2t[:, m:m + 1],
                                             scale=1.0, alpha=alpha)
                # q_in/16 = ae/32 + pe_q/32 (+ b3/32) ; qin reuses h1q slots
                qin = [qsb.tile([128, QR], F32R, tag="h1qin", name=f"qin{m}")
                       for m in range(2)]
                for m in range(2):
                    for n in range(QB):
                        ps = qps.tile([128, 512], F32, tag="qmlp", name="qmlp_ps")
                        for k in range(2):
                            nc.tensor.matmul(ps[:], w3q[k][:, m * 128:(m + 1) * 128],
                                             h2q[k][:, n * 512:(n + 1) * 512],
                                             start=(k == 0), stop=(k == 1))
                        sl = slice(n * 512, (n + 1) * 512)
                        if b3_zero:
                            nc.vector.tensor_add(qin[m][:, sl], ps[:], pq32[m][:, sl])
                        else:
                            tmp = qsb.tile([128, 512], F32, tag="aeqtmp",
                                           name="aeqtmp", bufs=3)
                            nc.scalar.activation(tmp[:], ps[:], AF.Identity,
                                                 bias=b3qt[:, m:m + 1], scale=1.0)
                            nc.vector.tensor_add(qin[m][:, sl], tmp[:], pq32[m][:, sl])
                # qT = (q_in/16) @ (Wq+I)
                for m in range(2):
                    for n in range(QB):
                        ps = qps.tile([128, 512], F32, tag="qmlp", name="qmlp_ps")
                        for k in range(2):
                            nc.tensor.matmul(ps[:], wqp[k][:, m * 128:(m + 1) * 128],
                                             qin[k][:, n * 512:(n + 1) * 512],
                                             start=(k == 0), stop=(k == 1))
                        nc.scalar.activation(qT[m][:, n * 512:(n + 1) * 512], ps[:],
                                             AF.Copy, bias=0.0, scale=1.0)

            # ---------------- main loop over q-tiles ----------------
            with tc.tile_pool(name="sS", bufs=2) as sS, \
                 tc.tile_pool(name="sM", bufs=2) as sM, \
                 tc.tile_pool(name="sC", bufs=2) as sC, \
                 tc.tile_pool(name="sO", bufs=4) as sO, \
                 tc.tile_pool(name="psS", bufs=2, space="PSUM") as psS, \
                 tc.tile_pool(name="psA", bufs=2, space="PSUM") as psA:
                for rep in range(main_reps):
                  for t in range(N_TILES):
                    tsl = slice(t * 128, (t + 1) * 128)
                    # --- S = pe_sims tile [128, 4096] fp32 ---
                    S = sS.tile([128, NK], F32, tag="S", name="S")
                    for g in range(4):
                        ps = psS.tile([128, 1024], F32, tag="psS", name="psS")
                        for h in range(2):
                            for k in range(2):
                                nc.tensor.matmul(
                                    ps[:, h * 512:(h + 1) * 512],
                                    pq8[k][:, tsl],
                                    pekv_h[k][:, (2 * g + h) * 512:(2 * g + h + 1) * 512],
                                    start=(k == 0), stop=(k == 1))
                        nc.scalar.activation(S[:, g * 1024:(g + 1) * 1024], ps[:],
                                             AF.Copy, bias=0.0, scale=1.0)

                    # --- topk threshold ---
                    cand = sC.tile([128, 8 * NCH], F32, tag="cand", name="cand")
                    for c in range(NCH):
                        nc.vector.max(out=cand[:, c * 8:(c + 1) * 8],
                                      in_=S[:, c * CHUNK:(c + 1) * CHUNK])
                    work = sC.tile([128, 8 * NCH], F32, tag="work", name="work")
                    m8 = sC.tile([128, 8], F32, tag="m8", name="m8")
                    src = cand
                    for r in range(TOP_K // 8 - 1):
                        nc.vector.max(out=m8[:], in_=src[:])
                        nc.vector.match_replace(out=work[:], in_to_replace=m8[:],
                                                in_values=src[:], imm_value=NEG)
                        src = work
                    vhat = sC.tile([128, 8], F32, tag="vhat", name="vhat")
                    nc.vector.max(out=vhat[:], in_=src[:])
                    # mask = S >= vhat[:, 7]
                    msk = sM.tile([128, NK], F32, tag="msk", name="msk")
                    nc.vector.tensor_scalar(msk[:], S[:], vhat[:, 7:8], None,
                                            op0=mybir.AluOpType.is_ge)

                    # --- att tile + mask-multiply + store ---
                    for g in range(4):
                        ps = psA.tile([128, 1024], F32, tag="psA", name="psA")
                        for h in range(2):
                            for k in range(2):
                                nc.tensor.matmul(
                                    ps[:, h * 512:(h + 1) * 512],
                                    qT[k][:, tsl],
                                    keyT[k][:, (2 * g + h) * 512:(2 * g + h + 1) * 512],
                                    start=(k == 0), stop=(k == 1))
                        ob = sO.tile([128, 1024], F32, tag="ob", name="ob")
                        nc.vector.tensor_mul(ob[:], ps[:], msk[:, g * 1024:(g + 1) * 1024])
                        nc.sync.dma_start(out.ap()[tsl, g * 1024:(g + 1) * 1024], ob[:])

    nc.compile()
    return nc


def _in_maps(inputs):
    f32 = np.float32
    feat_q = np.ascontiguousarray(inputs["feat_q"], dtype=f32)
    pe_q = np.ascontiguousarray(inputs["pe_q"], dtype=f32)
    feat_kv = np.ascontiguousarray(inputs["feat_kv"], dtype=f32)
    pe_kv = np.ascontiguousarray(inputs["pe_kv"], dtype=f32)
    W1 = np.ascontiguousarray(inputs["W1"], dtype=f32)
    W2 = np.ascontiguousarray(inputs["W2"], dtype=f32)
    W3 = np.asarray(inputs["W3"], dtype=f32)
    Wq = np.asarray(inputs["Wq"], dtype=f32)
    Wk = np.asarray(inputs["Wk"], dtype=f32)
    b1 = np.asarray(inputs["b1"], dtype=f32)
    b2 = np.asarray(inputs["b2"], dtype=f32)
    b3 = np.asarray(inputs["b3"], dtype=f32)
    eye = np.eye(D_MODEL, dtype=f32)

    def pack_bias(b):
        return np.ascontiguousarray(b.reshape(2, 128).T)

    shared = {
        "feat_kv": feat_kv,
        "pe_kv": pe_kv,
        "W1": W1,
        "W2": W2,
        "W3kv": np.ascontiguousarray(0.5 * W3),
        "W3q": np.ascontiguousarray(W3 / 32.0),
        "Wqp": np.ascontiguousarray(Wq + eye),
        "Wkp": np.ascontiguousarray(Wk + eye),
        "b1": pack_bias(b1),
        "b2": pack_bias(b2),
        "b3kv": pack_bias(0.5 * b3),
        "b3q": pack_bias(b3 / 32.0),
    }
    maps = []
    for c in range(N_CORES):
        m = dict(shared)
        m["feat_q"] = np.ascontiguousarray(feat_q[c * QR:(c + 1) * QR])
        m["pe_q"] = np.ascontiguousarray(pe_q[c * QR:(c + 1) * QR])
        maps.append(m)
    return maps


def get_nc(alpha: float, b3_zero: bool, main_reps: int = 1):
    key = (float(alpha), bool(b3_zero), int(main_reps))
    if key not in _CACHE:
        _CACHE[key] = _build(*key)
    return _CACHE[key]


def kernel(**inputs) -> np.ndarray:
    alpha = float(np.asarray(inputs["alpha"]))
    b3_zero = not np.any(np.asarray(inputs["b3"]))
    nc = get_nc(alpha, b3_zero)
    maps = _in_maps(inputs)
    res = run_bass_kernel_spmd(nc, maps, core_ids=list(range(N_CORES)))
    return np.concatenate([r["out"] for r in res.results], axis=0)

